# revision 1
# baseline (speedup 1.0000x reference)
"""CrossAttnBlock TRN2 kernel: 8-way (batch x l-half) sharded, collective-free.

Reference math (b=4, c=64, h=64, w=32, dim=256, HEADS=8, l=h*w=2048):
  zf = z.reshape(b, dim, l).T            # [b, l, dim]
  q  = x.reshape(b, c, l).T              # [b, l, c]
  k  = (zf @ Wk + bk) -> [b, H, l, c];  v likewise
  S  = q @ k.T / sqrt(c); A = softmax(S, -1); P = A @ v
  out = (P heads-concat) @ Wo + bo       # [b, l, c]
  return x + out.reshape(b, c, h, w)     # raw-memory reinterpretation

Per-core (core = bi*2 + half): full K/V projection for batch bi, attention +
out-proj for l rows [half*1024, (half+1)*1024). All layouts fall out of raw
input memory: z raw = zf^T ([dim, l]), x raw = q^T ([c, l]), out rows = raw
flat output. Scores are computed transposed (S^T [m, l]) so the AV contraction
runs with m on partitions; softmax denominators come from a ones-augmented V
column; normalization is applied per-head to the [l, c]-layout out-proj
partials where the divisor is a per-partition scalar.
"""
import ml_dtypes
import numpy as np

import concourse.bass as bass
import concourse.mybir as mybir
import concourse.tile as tile
from concourse import bacc
from concourse.bass_utils import run_bass_kernel_spmd
from concourse.masks import make_identity

F32 = mybir.dt.float32
F32R = mybir.dt.float32r
BF16 = mybir.dt.bfloat16

B, C, H, W = 4, 64, 64, 32
DIM = 256
HEADS = 8
L = H * W            # 2048
LH = L // 2          # 1024 per core
INNER = HEADS * C    # 512
N_CORES = 8

_CACHE = {}


def _r(ap):
    return ap.bitcast(F32R) if ap.dtype == F32 else ap


def build_nc():
    nc = bacc.Bacc("TRN2", target_bir_lowering=False, debug=False,
                   num_devices=N_CORES)
    xq = nc.dram_tensor("xq", [C, LH], BF16, kind="ExternalInput")
    xr = nc.dram_tensor("xr", [128, LH // 128, C], F32, kind="ExternalInput")
    zb = nc.dram_tensor("zb", [DIM, L], BF16, kind="ExternalInput")
    Wk = nc.dram_tensor("Wk", [DIM, INNER], BF16, kind="ExternalInput")
    Wv = nc.dram_tensor("Wv", [DIM, INNER], BF16, kind="ExternalInput")
    Wo = nc.dram_tensor("Wo", [C, HEADS, C], BF16, kind="ExternalInput")
    bk = nc.dram_tensor("bk", [128, 4], F32, kind="ExternalInput")
    bv = nc.dram_tensor("bv", [1, INNER], BF16, kind="ExternalInput")
    bo = nc.dram_tensor("bo", [1, C], BF16, kind="ExternalInput")
    ones_b = nc.dram_tensor("ones_b", [128, 128], BF16, kind="ExternalInput")
    OUT = nc.dram_tensor("out", [LH, C], F32, kind="ExternalOutput")

    NMT = L // 128       # 16 m-tiles
    NLS = LH // 128      # 8 l-subtiles

    with tile.TileContext(nc) as tc:
        with (
            tc.tile_pool(name="const", bufs=1) as cp,
            tc.tile_pool(name="pexp", bufs=3) as pe_pool,
            tc.tile_pool(name="small", bufs=3) as sp,
            tc.tile_pool(name="ps_proj", bufs=2, space="PSUM") as ps_proj,
            tc.tile_pool(name="ps_s", bufs=2, space="PSUM") as ps_s,
            tc.tile_pool(name="ps_pt", bufs=1, space="PSUM") as ps_pt,
        ):
            # ---- constants / inputs in SBUF ----
            z_sb = [cp.tile([128, L], BF16, tag=f"z{d}", name=f"z{d}") for d in range(2)]
            for d in range(2):
                nc.sync.dma_start(out=z_sb[d], in_=zb[d * 128:(d + 1) * 128, :])
            wk_sb = [cp.tile([128, INNER], BF16, tag=f"wk{d}", name=f"wk{d}") for d in range(2)]
            wv_sb = [cp.tile([128, INNER], BF16, tag=f"wv{d}", name=f"wv{d}") for d in range(2)]
            for d in range(2):
                nc.sync.dma_start(out=wk_sb[d], in_=Wk[d * 128:(d + 1) * 128, :])
                nc.sync.dma_start(out=wv_sb[d], in_=Wv[d * 128:(d + 1) * 128, :])
            wo_sb = cp.tile([C, HEADS, C], BF16, tag="wo")
            nc.sync.dma_start(out=wo_sb, in_=Wo[:, :, :])
            x_sb = cp.tile([128, LH], BF16, tag="x")
            nc.sync.dma_start(out=x_sb[0:C, :], in_=xq[:, :])
            nc.sync.dma_start(out=x_sb[C:2 * C, :], in_=xq[:, :])
            xr_sb = cp.tile([128, NLS, C], F32, tag="xr")
            nc.sync.dma_start(out=xr_sb, in_=xr[:, :, :])
            bk_sb = cp.tile([128, 4], F32, tag="bk")
            nc.sync.dma_start(out=bk_sb, in_=bk[:, :])
            bv_sb = cp.tile([1, INNER], BF16, tag="bv")
            nc.sync.dma_start(out=bv_sb, in_=bv[:, :])
            bo_sb = cp.tile([1, C], BF16, tag="bo")
            nc.sync.dma_start(out=bo_sb, in_=bo[:, :])
            ones_bf = cp.tile([1, 128], BF16, tag="ones_bf")
            nc.sync.dma_start(out=ones_bf, in_=ones_b[0:1, :])
            ident = cp.tile([8, 8], BF16, tag="ident")
            make_identity(nc, ident)

            kT_sb = [cp.tile([128, L], BF16, tag=f"kT{t}", name=f"kT{t}") for t in range(4)]
            v_sb = cp.tile([128, NMT, HEADS, C + 1], BF16, tag="v")
            nc.sync.dma_start(
                out=v_sb[:, :, :, C:C + 1],
                in_=ones_b.rearrange("p (a b c) -> p a b c", a=NMT, b=HEADS))
            pt_sb = [cp.tile([C + 1, LH], BF16, tag=f"pt{h}", name=f"pt{h}") for h in range(HEADS)]
            sums_sb = cp.tile([HEADS, LH], BF16, tag="sums")

            # ---- Phase A: kT[ci, m] = (Wk^T @ zf^T) + bk ----
            for t in range(4):
                for s in range(4):        # m slice (512 wide)
                    pk = ps_proj.tile([128, 512], F32, tag="proj")
                    for d in range(2):
                        nc.tensor.matmul(
                            pk,
                            wk_sb[d][:, t * 128:(t + 1) * 128],
                            z_sb[d][:, s * 512:(s + 1) * 512],
                            start=(d == 0), stop=(d == 1))
                    nc.vector.tensor_scalar(
                        out=kT_sb[t][:, s * 512:(s + 1) * 512], in0=pk,
                        scalar1=bk_sb[:, t:t + 1], scalar2=None,
                        op0=mybir.AluOpType.add)
            # ---- Phase B: v[m, ci] = zf @ Wv + bv (ones col appended) ----
            for s in range(NMT):          # m tile (128 rows)
                pv = ps_proj.tile([128, 512], F32, tag="proj")
                for d in range(2):
                    nc.tensor.matmul(
                        pv,
                        z_sb[d][:, s * 128:(s + 1) * 128],
                        wv_sb[d],
                        start=(d == 0), stop=False)
                nc.tensor.matmul(pv, ones_bf, bv_sb, start=False, stop=True)
                nc.vector.tensor_copy(
                    out=v_sb[:, s, :, 0:C],
                    in_=pv.rearrange("p (h c) -> p h c", h=HEADS))

            # ---- Phase C: attention per head ----
            for h in range(HEADS):
                t, roff = h // 2, 64 * (h % 2)
                ptp = ps_pt.tile([C + 1, LH], F32, tag="pt")
                for mt in range(NMT):
                    pss = ps_s.tile([128, LH], F32, tag="s")
                    for lh_ in range(2):
                        nc.tensor.matmul(
                            pss[:, lh_ * 512:(lh_ + 1) * 512],
                            kT_sb[t][roff:roff + 64, mt * 128:(mt + 1) * 128],
                            x_sb[roff:roff + C, lh_ * 512:(lh_ + 1) * 512],
                            start=True, stop=True)
                    es = pe_pool.tile([128, LH], BF16, tag="es")
                    nc.scalar.activation(out=es, in_=pss,
                                         func=mybir.ActivationFunctionType.Exp,
                                         scale=float(C) ** -0.5)
                    for lh_ in range(2):
                        nc.tensor.matmul(
                            ptp[:, lh_ * 512:(lh_ + 1) * 512],
                            v_sb[:, mt, h, :],
                            es[:, lh_ * 512:(lh_ + 1) * 512],
                            start=(mt == 0), stop=(mt == NMT - 1))
                nc.vector.tensor_copy(out=pt_sb[h], in_=ptp)
                nc.sync.dma_start(out=sums_sb[h:h + 1, :],
                                  in_=pt_sb[h][C:C + 1, :])

            # ---- Phase D: out-proj + normalize + residual per l-subtile ----
            for ls in range(NLS):
                ptr = ps_proj.tile([128, 8], BF16, tag="proj")
                nc.tensor.transpose(ptr, sums_sb[:, ls * 128:(ls + 1) * 128], ident)
                recip = sp.tile([128, 8], F32, tag="recip")
                nc.vector.reciprocal(out=recip, in_=ptr)
                acc = None
                for h in range(HEADS):
                    po = ps_proj.tile([128, C], F32, tag="proj")
                    nc.tensor.matmul(
                        po,
                        pt_sb[h][0:C, ls * 128:(ls + 1) * 128],
                        wo_sb[:, h, :],
                        start=True, stop=(h != 0))
                    if h == 0:
                        nc.tensor.matmul(po, ones_bf, bo_sb,
                                         start=False, stop=True)
                    tmp = sp.tile([128, C], F32, tag="tmp")
                    nc.vector.tensor_scalar(
                        out=tmp, in0=po, scalar1=recip[:, h:h + 1],
                        scalar2=None, op0=mybir.AluOpType.mult)
                    if h == 0:
                        acc = sp.tile([128, C], F32, tag="oacc")
                        nc.vector.tensor_tensor(
                            out=acc, in0=xr_sb[:, ls, :], in1=tmp,
                            op=mybir.AluOpType.add)
                    else:
                        nc.vector.tensor_tensor(
                            out=acc, in0=acc, in1=tmp,
                            op=mybir.AluOpType.add)
                nc.sync.dma_start(out=OUT[ls * 128:(ls + 1) * 128, :], in_=acc)

    nc.compile()
    return nc


def kernel(x, z, Wk, bk, Wv, bv, Wo, bo):
    x = np.ascontiguousarray(x, dtype=np.float32)
    z = np.ascontiguousarray(z, dtype=np.float32)
    if "nc" not in _CACHE:
        _CACHE["nc"] = build_nc()
    nc = _CACHE["nc"]
    shared = {
        "Wk": np.ascontiguousarray(np.asarray(Wk, np.float32).astype(ml_dtypes.bfloat16)),
        "Wv": np.ascontiguousarray(np.asarray(Wv, np.float32).astype(ml_dtypes.bfloat16)),
        "Wo": np.ascontiguousarray(np.asarray(Wo, np.float32)
                                   .reshape(HEADS, C, C).transpose(1, 0, 2)
                                   .astype(ml_dtypes.bfloat16)),
        "bk": np.ascontiguousarray(
            np.asarray(bk, np.float32).reshape(4, 128).T),
        "bv": np.ascontiguousarray(
            np.asarray(bv, np.float32).reshape(1, INNER).astype(ml_dtypes.bfloat16)),
        "bo": np.ascontiguousarray(
            np.asarray(bo, np.float32).reshape(1, C).astype(ml_dtypes.bfloat16)),
        "ones_b": np.ones((128, 128), ml_dtypes.bfloat16),
    }
    in_maps = []
    for core in range(N_CORES):
        bi, half = core // 2, core % 2
        xi = x[bi].reshape(C, L)
        in_maps.append({
            "xq": np.ascontiguousarray(
                xi[:, half * LH:(half + 1) * LH].astype(ml_dtypes.bfloat16)),
            "xr": np.ascontiguousarray(
                x[bi].reshape(-1)[half * LH * C:(half + 1) * LH * C]
                .reshape(LH // 128, 128, C).transpose(1, 0, 2)),
            "zb": np.ascontiguousarray(
                z[bi].reshape(DIM, L).astype(ml_dtypes.bfloat16)),
            **shared,
        })
    _CACHE["in_maps"] = in_maps
    res = run_bass_kernel_spmd(nc, in_maps, list(range(N_CORES)))
    full = np.empty((B, L * C), dtype=np.float32)
    for core in range(N_CORES):
        bi, half = core // 2, core % 2
        full[bi, half * LH * C:(half + 1) * LH * C] = \
            res.results[core]["out"].reshape(-1)
    return full.reshape(B, C, H, W)



# revision 3
# speedup vs baseline: 1.0258x; 1.0258x over previous
"""CrossAttnBlock TRN2 kernel: 8-way (batch x l-half) sharded, collective-free.

Reference math (b=4, c=64, h=64, w=32, dim=256, HEADS=8, l=h*w=2048):
  zf = z.reshape(b, dim, l).T            # [b, l, dim]
  q  = x.reshape(b, c, l).T              # [b, l, c]
  k  = (zf @ Wk + bk) -> [b, H, l, c];  v likewise
  S  = q @ k.T / sqrt(c); A = softmax(S, -1); P = A @ v
  out = (P heads-concat) @ Wo + bo       # [b, l, c]
  return x + out.reshape(b, c, h, w)     # raw-memory reinterpretation

Per-core (core = bi*2 + half): full K/V projection for batch bi, attention +
out-proj for l rows [half*1024, (half+1)*1024).

v2 optimizations over the first working version:
 - bk dropped entirely (a per-row additive constant cancels in softmax);
   bv/bo folded on the host into the residual input (softmax rows sum to 1,
   so A @ (V + 1 bv^T) @ Wo + bo = A@V@Wo + (bv@Wo + bo)).
 - Score matmuls for the two heads sharing a kT tile are interleaved so the
   row-group-0 (partitions 0-63) and row-group-64 matmuls run concurrently
   on the PE array (contraction is only c=64).
 - The softmax exp is split between ScalarE (exact table exp) and VectorE
   (Schraudolph bit-trick exp: es_bf16_bits = int16(S*A + B), exploiting the
   f32->bf16 bit layout; softmax self-normalization cancels the ~3% element
   error to ~1e-5 at the output).
 - Phase D normalize+accumulate fused into one scalar_tensor_tensor per head.
 - Projection PSUM->SBUF copies merged into [128,1024] ops and spread across
   VectorE and ScalarE; input DMAs spread across engine queues.
"""
import ml_dtypes
import numpy as np

import concourse.bass as bass
import concourse.mybir as mybir
import concourse.tile as tile
from concourse import bacc
from concourse.bass_utils import run_bass_kernel_spmd
from concourse.masks import make_identity

F32 = mybir.dt.float32
BF16 = mybir.dt.bfloat16
I16 = mybir.dt.int16

B, C, H, W = 4, 64, 64, 32
DIM = 256
HEADS = 8
L = H * W            # 2048
LH = L // 2          # 1024 per core
INNER = HEADS * C    # 512
N_CORES = 8

# Schraudolph exp constants (exp(x/8) via int16 bitcast to bf16), tuned for
# min worst-case relative error (~3.3%) incl. trunc-toward-zero compensation.
EXP_A = 23.083120654223414
EXP_B = 16250.868

_CACHE = {}


def _dve_exp(h, mt):
    # ~43 of 128 exp tiles on VectorE, rest on ScalarE (load balance)
    return (h + mt) % 3 == 1


def build_nc():
    nc = bacc.Bacc("TRN2", target_bir_lowering=False, debug=False,
                   num_devices=N_CORES)
    xq = nc.dram_tensor("xq", [C, LH], BF16, kind="ExternalInput")
    xr = nc.dram_tensor("xr", [128, LH // 128, C], F32, kind="ExternalInput")
    zb = nc.dram_tensor("zb", [DIM, L], BF16, kind="ExternalInput")
    Wk = nc.dram_tensor("Wk", [DIM, INNER], BF16, kind="ExternalInput")
    Wv = nc.dram_tensor("Wv", [DIM, INNER], BF16, kind="ExternalInput")
    Wo = nc.dram_tensor("Wo", [C, HEADS, C], BF16, kind="ExternalInput")
    OUT = nc.dram_tensor("out", [LH, C], F32, kind="ExternalOutput")

    NMT = L // 128       # 16 m-tiles
    NLS = LH // 128      # 8 l-subtiles

    with tile.TileContext(nc) as tc:
        with (
            tc.tile_pool(name="const", bufs=1) as cp,
            tc.tile_pool(name="pexp", bufs=2) as pe_pool,
            tc.tile_pool(name="small", bufs=3) as sp,
            tc.tile_pool(name="ps", bufs=1, space="PSUM") as ps,
        ):
            # ---- constants / inputs in SBUF (DMAs spread across queues) ----
            z_sb = [cp.tile([128, L], BF16, tag=f"z{d}", name=f"z{d}") for d in range(2)]
            nc.sync.dma_start(out=z_sb[0], in_=zb[0:128, :])
            nc.scalar.dma_start(out=z_sb[1], in_=zb[128:256, :])
            wk_sb = [cp.tile([128, INNER], BF16, tag=f"wk{d}", name=f"wk{d}") for d in range(2)]
            wv_sb = [cp.tile([128, INNER], BF16, tag=f"wv{d}", name=f"wv{d}") for d in range(2)]
            for d in range(2):
                nc.gpsimd.dma_start(out=wk_sb[d], in_=Wk[d * 128:(d + 1) * 128, :])
                nc.sync.dma_start(out=wv_sb[d], in_=Wv[d * 128:(d + 1) * 128, :])
            wo_sb = cp.tile([C, HEADS, C], BF16, tag="wo")
            nc.gpsimd.dma_start(out=wo_sb, in_=Wo[:, :, :])
            x_sb = cp.tile([128, LH], BF16, tag="x")
            nc.gpsimd.dma_start(out=x_sb[0:C, :], in_=xq[:, :])
            nc.gpsimd.dma_start(out=x_sb[C:2 * C, :], in_=xq[:, :])
            xr_sb = cp.tile([128, NLS, C], F32, tag="xr")
            nc.gpsimd.dma_start(out=xr_sb, in_=xr[:, :, :])
            ident = cp.tile([8, 8], BF16, tag="ident")
            make_identity(nc, ident)

            kT_sb = [cp.tile([128, L], BF16, tag=f"kT{t}", name=f"kT{t}") for t in range(4)]
            v_sb = cp.tile([128, NMT, HEADS, C + 1], BF16, tag="v")
            nc.vector.memset(v_sb[:, :, :, C:C + 1], 1.0)
            pt_sb = [cp.tile([C + 1, LH], BF16, tag=f"pt{h}", name=f"pt{h}") for h in range(HEADS)]
            sums_sb = cp.tile([HEADS, LH], BF16, tag="sums")

            # ---- Phase A: kT[ci, m] = Wk^T @ zf^T ----
            for t in range(4):
                for s2 in range(2):       # 1024-wide m slices
                    pk = ps.tile([128, 1024], F32, tag=("sA" if (t + s2) % 2 == 0 else "sB"), name="pk")
                    for half in range(2):
                        for d in range(2):
                            nc.tensor.matmul(
                                pk[:, half * 512:(half + 1) * 512],
                                wk_sb[d][:, t * 128:(t + 1) * 128],
                                z_sb[d][:, s2 * 1024 + half * 512:s2 * 1024 + (half + 1) * 512],
                                start=(d == 0), stop=(d == 1))
                    dst = kT_sb[t][:, s2 * 1024:(s2 + 1) * 1024]
                    if (t + s2) % 2 == 0:
                        nc.vector.tensor_copy(out=dst, in_=pk)
                    else:
                        nc.scalar.copy(out=dst, in_=pk)
            # ---- Phase B: v[m, (h c)] = zf @ Wv (ones col preset) ----
            for s2 in range(NMT // 2):    # 2 m-tiles per PSUM tile
                pv = ps.tile([128, 1024], F32, tag=("sA" if s2 % 2 == 0 else "sB"), name="pv")
                for half in range(2):
                    mt = 2 * s2 + half
                    for d in range(2):
                        nc.tensor.matmul(
                            pv[:, half * 512:(half + 1) * 512],
                            z_sb[d][:, mt * 128:(mt + 1) * 128],
                            wv_sb[d],
                            start=(d == 0), stop=(d == 1))
                dst = v_sb[:, 2 * s2:2 * s2 + 2, :, 0:C]
                src = pv.rearrange("p (m h c) -> p m h c", m=2, h=HEADS)
                if s2 % 2 == 0:
                    nc.vector.tensor_copy(out=dst, in_=src)
                else:
                    nc.scalar.copy(out=dst, in_=src)

            # ---- Phase C: attention, head pairs interleaved on PE row groups ----
            for t in range(4):
                h0, h1 = 2 * t, 2 * t + 1
                ptp = [ps.tile([C + 1, LH], F32, tag=f"pt{i}", name=f"ptp{i}")
                       for i in range(2)]
                for mt in range(NMT):
                    pss = [ps.tile([128, LH], F32, tag=tg, name=f"pss_{tg}")
                           for tg in ("sA", "sB")]
                    for lh_ in range(2):
                        for i, roff in ((0, 0), (1, 64)):
                            nc.tensor.matmul(
                                pss[i][:, lh_ * 512:(lh_ + 1) * 512],
                                kT_sb[t][roff:roff + 64, mt * 128:(mt + 1) * 128],
                                x_sb[roff:roff + C, lh_ * 512:(lh_ + 1) * 512],
                                start=True, stop=True)
                    es = [pe_pool.tile([128, LH], BF16, tag=tg, name=f"es_{tg}")
                          for tg in ("esA", "esB")]
                    for i, h in ((0, h0), (1, h1)):
                        if _dve_exp(h, mt):
                            nc.vector.tensor_scalar(
                                out=es[i].bitcast(I16), in0=pss[i],
                                scalar1=EXP_A, scalar2=EXP_B,
                                op0=mybir.AluOpType.mult,
                                op1=mybir.AluOpType.add)
                        else:
                            nc.scalar.activation(
                                out=es[i], in_=pss[i],
                                func=mybir.ActivationFunctionType.Exp,
                                scale=float(C) ** -0.5)
                    for i, h in ((0, h0), (1, h1)):
                        for lh_ in range(2):
                            nc.tensor.matmul(
                                ptp[i][:, lh_ * 512:(lh_ + 1) * 512],
                                v_sb[:, mt, h, :],
                                es[i][:, lh_ * 512:(lh_ + 1) * 512],
                                start=(mt == 0), stop=(mt == NMT - 1))
                for i, h in ((0, h0), (1, h1)):
                    if i == 0:
                        nc.vector.tensor_copy(out=pt_sb[h], in_=ptp[i][0:C + 1, :])
                    else:
                        nc.scalar.copy(out=pt_sb[h], in_=ptp[i][0:C + 1, :])
                    nc.sync.dma_start(out=sums_sb[h:h + 1, :],
                                      in_=pt_sb[h][C:C + 1, :])

            # ---- Phase D: out-proj + normalize + residual per l-subtile ----
            for ls in range(NLS):
                ptr = ps.tile([128, 8], BF16, tag="pt0", name="ptr")
                nc.tensor.transpose(ptr, sums_sb[:, ls * 128:(ls + 1) * 128], ident)
                recip = sp.tile([128, 8], F32, tag="recip")
                nc.vector.reciprocal(out=recip, in_=ptr)
                acc = sp.tile([128, C], F32, tag="oacc")
                for h in range(HEADS):
                    po = ps.tile([128, C], F32, tag=("sA" if h % 2 == 0 else "sB"), name="po")
                    nc.tensor.matmul(
                        po,
                        pt_sb[h][0:C, ls * 128:(ls + 1) * 128],
                        wo_sb[:, h, :],
                        start=True, stop=True)
                    nc.vector.scalar_tensor_tensor(
                        out=acc, in0=po, scalar=recip[:, h:h + 1],
                        in1=(xr_sb[:, ls, :] if h == 0 else acc),
                        op0=mybir.AluOpType.mult,
                        op1=mybir.AluOpType.add)
                nc.sync.dma_start(out=OUT[ls * 128:(ls + 1) * 128, :], in_=acc)

    nc.compile()
    return nc


def kernel(x, z, Wk, bk, Wv, bv, Wo, bo):
    x = np.ascontiguousarray(x, dtype=np.float32)
    z = np.ascontiguousarray(z, dtype=np.float32)
    if "nc" not in _CACHE:
        _CACHE["nc"] = build_nc()
    nc = _CACHE["nc"]
    # bv/bo fold into a constant output row (softmax rows sum to 1); bk
    # cancels inside the softmax.
    bconst = (np.asarray(bv, np.float32) @ np.asarray(Wo, np.float32)
              + np.asarray(bo, np.float32))           # [C]
    shared = {
        "Wk": np.ascontiguousarray(np.asarray(Wk, np.float32).astype(ml_dtypes.bfloat16)),
        "Wv": np.ascontiguousarray(np.asarray(Wv, np.float32).astype(ml_dtypes.bfloat16)),
        "Wo": np.ascontiguousarray(np.asarray(Wo, np.float32)
                                   .reshape(HEADS, C, C).transpose(1, 0, 2)
                                   .astype(ml_dtypes.bfloat16)),
    }
    in_maps = []
    for core in range(N_CORES):
        bi, half = core // 2, core % 2
        xi = x[bi].reshape(C, L)
        in_maps.append({
            "xq": np.ascontiguousarray(
                xi[:, half * LH:(half + 1) * LH].astype(ml_dtypes.bfloat16)),
            "xr": np.ascontiguousarray(
                (x[bi].reshape(-1)[half * LH * C:(half + 1) * LH * C]
                 .reshape(LH // 128, 128, C) + bconst)
                .transpose(1, 0, 2)),
            "zb": np.ascontiguousarray(
                z[bi].reshape(DIM, L).astype(ml_dtypes.bfloat16)),
            **shared,
        })
    _CACHE["in_maps"] = in_maps
    res = run_bass_kernel_spmd(nc, in_maps, list(range(N_CORES)))
    full = np.empty((B, L * C), dtype=np.float32)
    for core in range(N_CORES):
        bi, half = core // 2, core % 2
        full[bi, half * LH * C:(half + 1) * LH * C] = \
            res.results[core]["out"].reshape(-1)
    return full.reshape(B, C, H, W)
